# revision 30
# baseline (speedup 1.0000x reference)
"""GCNConv (PyG-faithful, normalize=True, add_self_loops=True) on 8 Trainium2
NeuronCores via Bass/Tile.

Strategy (1D graph/data parallel, v2):
  - Nodes partitioned across 8 cores (12500 rows each, padded to 12544 = 98
    blocks of 128).
  - Phase A: each core computes h = (x @ W) * dinv_row in bf16 and writes it
    out in four "quarter" pieces (25/25/24/24 blocks). After each quarter an
    AllGather ships that quarter of every core into a per-quarter shared
    table chunk, so phase B can start gathering from chunk 0 while chunks
    1-3 are still in flight.
  - Phase B: each core owns 1/8 of the destinations. Edges (self-loops
    excluded - they are folded into the epilogue) are host-sorted by
    (dst-window, src-quarter, dst-block) and packed densely per
    (window, quarter) section; trailing pad indices are -1, which the SWDGE
    gather ucode trims, so pads cost no DMA packets. Four SWDGE queues (one
    per source quarter) gather 256B bf16 rows; host-precomputed one-hot fp8
    sel tiles feed TensorE matmuls that segment-sum each destination
    block's messages in PSUM. Epilogue per block: add the self-loop term
    (own h row), scale by dinv_dst, add bias.
"""

import sys

if "/opt/trn_rl_repo" not in sys.path:
    sys.path.insert(0, "/opt/trn_rl_repo")

import numpy as np

P = 128
NCORES = 8
WBLK = 7          # dst blocks per window
NW = 14           # windows (98 blocks / 7)
Q_BLKS = [25, 25, 24, 24]          # blocks per source quarter
Q_OFF_B = [0, 25, 50, 74]          # quarter start block
N_NODES = 100000
NB = N_NODES // NCORES             # 12500
BLOCKS = 98
NBP = BLOCKS * P                   # 12544


def _pack(x, edge_index, weight, b):
    import ml_dtypes

    x = np.asarray(x, dtype=np.float32)
    ei = np.asarray(edge_index)
    weight = np.asarray(weight, dtype=np.float32)
    bias = np.asarray(b, dtype=np.float32).reshape(-1)

    n, nin = x.shape
    assert n == N_NODES and nin == P and weight.shape == (P, P)
    q_rows = [q * P for q in Q_BLKS]            # [3200,3200,3072,3072]
    q_off_r = [o * P for o in Q_OFF_B]          # row offsets within a shard
    chunk_rows = [NCORES * r for r in q_rows]

    src = ei[0].astype(np.int64)
    dst = ei[1].astype(np.int64)
    m = src.shape[0]

    deg = np.bincount(dst, minlength=n).astype(np.float32) + 1.0
    dinv = 1.0 / np.sqrt(deg)

    # --- per-edge coordinates (no self-loops; folded into epilogue)
    core = dst // NB
    dlc = dst - core * NB
    blk = dlc >> 7                   # dst block within core [0,98)
    dl = (dlc & 127).astype(np.int64)
    w = blk // WBLK                  # window [0,14)

    ks = src // NB
    ls = src - ks * NB
    sb = ls >> 7                     # src block within owner core
    q = np.digitize(sb, Q_OFF_B[1:])             # quarter 0..3
    rel = ks * np.take(q_rows, q) + (ls - np.take(q_off_r, q))
    assert rel.max() < 32768

    # order edges per core by (w, q, blk)
    sect = (w * 4 + q)               # (w,q) section id [0,56)
    key = (core * 56 + sect) * BLOCKS + blk
    order = np.argsort(key, kind="stable")
    karr = core[order]
    relarr = rel[order].astype(np.int16)
    dlarr = dl[order]
    sectarr = sect[order]
    blkarr = blk[order]

    # counts
    cnt_kwqb = np.bincount(
        (core * 56 + sect) * BLOCKS + blk, minlength=NCORES * 56 * BLOCKS
    ).reshape(NCORES, NW, 4, BLOCKS)
    cnt_kwq = cnt_kwqb.sum(axis=3)                       # [8, 14, 4]
    S_wq = -(-cnt_kwq.max(axis=0) // P)                  # [14, 4] tiles
    # per-core block start slot within its (w,q) section
    bstart = np.zeros((NCORES, NW, 4, BLOCKS), np.int64)
    for wi in range(NW):
        bl = slice(wi * WBLK, (wi + 1) * WBLK)
        c = cnt_kwqb[:, wi, :, bl]
        bstart[:, wi, :, bl] = np.cumsum(c, axis=2) - c

    # global tile layout + matmul plan
    sec0 = np.zeros((NW, 4), np.int64)      # global tile offset per section
    O_wq = np.zeros((NW, 4), np.int64)      # msg slot-tile offset in window
    t_total = 0
    for wi in range(NW):
        off = 0
        for qi in range(4):
            sec0[wi, qi] = t_total
            O_wq[wi, qi] = off
            t_total += int(S_wq[wi, qi])
            off += int(S_wq[wi, qi])
    jmax = int(S_wq.sum(axis=1).max())

    # matmul plan: per (w,q) ordered list of (b, t); col index global
    mm_col_of = np.full((NW, 4, BLOCKS, int(S_wq.max()) + 1), -1, np.int64)
    plan = [[[] for _ in range(4)] for _ in range(NW)]   # [(b, t, col)]
    selcol0 = np.zeros((NW, 4), np.int64)
    mm_total = 0
    ntiles_b = np.zeros(BLOCKS, np.int64)
    for wi in range(NW):
        for qi in range(4):
            selcol0[wi, qi] = mm_total
            for bb in range(wi * WBLK, (wi + 1) * WBLK):
                c = cnt_kwqb[:, wi, qi, bb]
                if c.max() == 0:
                    continue
                s = bstart[:, wi, qi, bb]
                e = s + c
                T0 = int((s // P).min())
                T1 = int((-(-e // P)).max())
                for t in range(T0, T1):
                    mm_col_of[wi, qi, bb, t] = mm_total
                    plan[wi][qi].append((bb, t, mm_total))
                    ntiles_b[bb] += 1
                    mm_total += 1
    assert (ntiles_b > 0).all()
    selmax = int(
        max(len(plan[wi][qi]) for wi in range(NW) for qi in range(4))
    )

    # per-core packed arrays
    gs = np.zeros(NCORES * NW * 4 * BLOCKS, np.int64)
    cr = cnt_kwqb.reshape(-1)
    gs[1:] = np.cumsum(cr)[:-1]
    kk = (karr * 56 + sectarr) * BLOCKS + blkarr
    rank_in_b = np.arange(m, dtype=np.int64) - gs[kk]
    wa = sectarr // 4
    qa = sectarr % 4
    slot = (
        bstart[karr, wa, qa, blkarr] + rank_in_b
    )                                                  # slot within section
    tile_in_sec = slot >> 7
    part = slot & 127
    gslot = sec0[wa, qa] * P + slot                    # global slot

    # pad slots gather row 0 of their chunk (sel column is zero, so the
    # value is discarded). Gathering pads costs ~4% extra packets but keeps
    # the per-core descriptor count equal to the static num_idxs, which the
    # SWDGE ring accounting requires (num_idxs_reg must match the actual
    # count, and per-core runtime registers proved unschedulable safely).
    idx_lin = np.zeros((NCORES, t_total * P), np.int16)
    idx_lin[karr, gslot] = relarr

    mmi = mm_col_of[wa, qa, blkarr, tile_in_sec]
    assert (mmi >= 0).all()
    sel_u8 = np.zeros((NCORES, P, mm_total * P), np.uint8)
    sel_u8[karr, part, mmi * P + dlarr] = 0x38         # fp8e4m3 1.0
    sel_pack = sel_u8.view(ml_dtypes.float8_e4m3)

    # wrap-16 + replicate to 128 partitions
    l16 = t_total * P // 16
    idx_w = idx_lin.reshape(NCORES, l16, 16).transpose(0, 2, 1)
    idx_pack = np.ascontiguousarray(np.tile(idx_w, (1, NCORES, 1)))

    # per-core xt (bf16), dinv
    xt = np.zeros((NCORES, P, NBP), ml_dtypes.bfloat16)
    dinv_t = np.zeros((NCORES, P, BLOCKS), np.float32)
    for k in range(NCORES):
        xs = x[k * NB : (k + 1) * NB]
        xt[k, :, :NB] = xs.T.astype(ml_dtypes.bfloat16)
        dv = np.zeros(NBP, np.float32)
        dv[:NB] = dinv[k * NB : (k + 1) * NB]
        dinv_t[k] = dv.reshape(BLOCKS, P).T
    w_bf = np.ascontiguousarray(weight.astype(ml_dtypes.bfloat16))
    bias_rep = np.ascontiguousarray(np.tile(bias[None, :], (P, 1)))
    ident_u8 = np.zeros((P, P), np.uint8)
    ident_u8[np.arange(P), np.arange(P)] = 0x38       # fp8e4m3 identity
    ident = ident_u8.view(ml_dtypes.float8_e4m3)

    meta = dict(
        q_rows=q_rows, chunk_rows=chunk_rows, S_wq=S_wq, sec0=sec0, O_wq=O_wq,
        jmax=jmax, t_total=t_total, l16=l16, plan=plan, selcol0=selcol0,
        mm_total=mm_total, selmax=selmax, ntiles_b=ntiles_b,
    )
    in_maps = [
        {
            "xt": xt[k],
            "w_in": w_bf,
            "bias": bias_rep,
            "dinv": dinv_t[k],
            "idxp": idx_pack[k],
            "selp": sel_pack[k],
            "ident": ident,
        }
        for k in range(NCORES)
    ]
    return meta, in_maps


def _build_program(meta):
    from concourse import bass, bacc, mybir
    import concourse.tile as tile

    q_rows = meta["q_rows"]
    chunk_rows = meta["chunk_rows"]
    S_wq = meta["S_wq"]
    sec0 = meta["sec0"]
    O_wq = meta["O_wq"]
    jmax = meta["jmax"]
    l16 = meta["l16"]
    plan = meta["plan"]
    mm_total = meta["mm_total"]
    selmax = meta["selmax"]
    ntiles_b = meta["ntiles_b"]

    f32 = mybir.dt.float32
    bf16 = mybir.dt.bfloat16
    fp8 = mybir.dt.float8e4

    nc = bacc.Bacc(num_swdge_queues=4)
    xt_in = nc.declare_dram_parameter("xt", [P, NBP], bf16, isOutput=False)
    w_in = nc.declare_dram_parameter("w_in", [P, P], bf16, isOutput=False)
    bias_in = nc.declare_dram_parameter("bias", [P, P], f32, isOutput=False)
    dinv_in = nc.declare_dram_parameter("dinv", [P, BLOCKS], f32, isOutput=False)
    idx_in = nc.declare_dram_parameter("idxp", [P, l16], mybir.dt.int16, isOutput=False)
    sel_in = nc.declare_dram_parameter("selp", [P, mm_total * P], fp8, isOutput=False)
    ident_in = nc.declare_dram_parameter("ident", [P, P], fp8, isOutput=False)
    out_ext = nc.declare_dram_parameter("out", [NBP, P], f32, isOutput=True)

    hq = [nc.dram_tensor(f"h_q{q}", [q_rows[q], P], bf16) for q in range(4)]
    gq = [
        nc.dram_tensor(f"g_q{q}", [chunk_rows[q], P], bf16, addr_space="Shared")
        for q in range(4)
    ]

    # quarter row ranges within the shard, for gl (self-loop) loads
    q_off_r = [0]
    for q in range(3):
        q_off_r.append(q_off_r[-1] + q_rows[q])

    with tile.TileContext(nc) as tc:
        with (
            tc.tile_pool(name="const", bufs=1) as cpool,
            tc.tile_pool(name="work", bufs=4) as wpool,
            tc.tile_pool(name="msgp", bufs=2) as mpool,
            tc.tile_pool(name="selp", bufs=8) as spool,
            tc.tile_pool(name="glp", bufs=2) as gpool,
            tc.tile_pool(name="outp", bufs=2) as opool,
            tc.tile_pool(name="psA", bufs=2, space="PSUM") as psA,
            tc.tile_pool(name="psB", bufs=4, space="PSUM") as psB,
        ):
            # constants / metadata
            w_sb = cpool.tile([P, P], bf16, tag="w")
            nc.sync.dma_start(out=w_sb[:], in_=w_in[:])
            bias_sb = cpool.tile([P, P], f32, tag="bias")
            nc.sync.dma_start(out=bias_sb[:], in_=bias_in[:])
            dinv_sb = cpool.tile([P, BLOCKS], f32, tag="dinv")
            nc.sync.dma_start(out=dinv_sb[:], in_=dinv_in[:])
            ident_sb = cpool.tile([P, P], fp8, tag="ident")
            nc.sync.dma_start(out=ident_sb[:], in_=ident_in[:])
            idx_sb = cpool.tile([P, l16], mybir.dt.int16, tag="idx")
            for i in range(4):
                s = l16 // 4
                e = l16 if i == 3 else (i + 1) * s
                nc.sync.dma_start(out=idx_sb[:, i * s : e], in_=idx_in[:, i * s : e])


            # zero the two msg buffers once (pad slots are never gathered;
            # stale SBUF bits could be NaN in bf16 and 0*NaN would poison
            # the segment-sum matmuls)
            for _ in range(2):
                mz = mpool.tile([P, jmax, P], bf16, tag="msg")
                nc.vector.memset(mz[:], 0.0)

            # ---- phase A: h = (x @ W) * dinv, written per quarter, then
            # AllGather that quarter into the shared table chunk.
            for q in range(4):
                nblk = Q_BLKS[q]
                npc = -(-nblk // 6)
                lo = nblk // npc
                pieces = [lo + 1] * (nblk - lo * npc) + [lo] * (npc * (lo + 1) - nblk)
                assert sum(pieces) == nblk and max(pieces) <= 6
                loff = 0
                for pb in pieces:
                    gb0 = Q_OFF_B[q] + loff // P
                    xt_t = wpool.tile([P, 6 * P], bf16, tag="xt")
                    nc.sync.dma_start(
                        out=xt_t[:, : pb * P],
                        in_=xt_in[:, q_off_r[q] + loff : q_off_r[q] + loff + pb * P],
                    )
                    hbig = wpool.tile([P, 6, P], bf16, tag="hbig")
                    for j in range(pb):
                        ph = psA.tile([P, P], f32, tag="ph")
                        nc.tensor.matmul(
                            out=ph[:],
                            lhsT=xt_t[:, j * P : (j + 1) * P],
                            rhs=w_sb[:],
                            start=True,
                            stop=True,
                        )
                        gb = gb0 + j
                        nc.vector.tensor_scalar(
                            out=hbig[:, j, :],
                            in0=ph[:],
                            scalar1=dinv_sb[:, gb : gb + 1],
                            scalar2=None,
                            op0=mybir.AluOpType.mult,
                        )
                    nc.sync.dma_start(
                        out=hq[q][loff : loff + pb * P, :].rearrange(
                            "(j p) f -> p j f", p=P
                        ),
                        in_=hbig[:, :pb, :],
                    )
                    loff += pb * P
                nc.gpsimd.collective_compute(
                    "AllGather",
                    mybir.AluOpType.bypass,
                    replica_groups=[list(range(NCORES))],
                    ins=[hq[q][:]],
                    outs=[gq[q][:]],
                )

            # ---- phase B
            for w in range(NW):
                msg = mpool.tile([P, jmax, P], bf16, tag="msg")
                sels = [None] * 4
                for q in range(4):
                    swq = int(S_wq[w, q])
                    if swq == 0:
                        continue
                    s0 = int(sec0[w, q])
                    nc.gpsimd.dma_gather(
                        out_ap=msg[:, int(O_wq[w, q]) : int(O_wq[w, q]) + swq, :],
                        in_ap=gq[q][:],
                        idxs_ap=idx_sb[:, s0 * 8 : (s0 + swq) * 8],
                        num_idxs=swq * P,
                        num_idxs_reg=swq * P,
                        elem_size=P,
                        single_packet=False,
                        queue_num=q,
                    )
                    nmm = len(plan[w][q])
                    if nmm:
                        selw = spool.tile([P, selmax * P], fp8, tag="selw")
                        c0 = int(meta["selcol0"][w, q])
                        nc.scalar.dma_start(
                            out=selw[:, : nmm * P],
                            in_=sel_in[:, c0 * P : (c0 + nmm) * P],
                        )
                        sels[q] = selw

                # self-loop rows for this window (may straddle quarters)
                gl = gpool.tile([P, WBLK, P], bf16, tag="gl")
                r0, r1 = w * WBLK * P, (w + 1) * WBLK * P
                for q in range(4):
                    a = max(r0, q_off_r[q])
                    bnd = q_off_r[q] + q_rows[q]
                    bq = min(r1, bnd)
                    if a >= bq:
                        continue
                    j0 = (a - r0) // P
                    j1 = (bq - r0) // P
                    nc.sync.dma_start(
                        out=gl[:, j0:j1, :],
                        in_=hq[q][a - q_off_r[q] : bq - q_off_r[q], :].rearrange(
                            "(j p) f -> p j f", p=P
                        ),
                    )

                tmp_w = opool.tile([P, WBLK, P], f32, tag="tmpw")
                osb_w = opool.tile([P, WBLK, P], f32, tag="osbw")
                for j, bb in enumerate(range(w * WBLK, (w + 1) * WBLK)):
                    nt = int(ntiles_b[bb])
                    acc = psB.tile([P, P], f32, tag="acc")
                    ti = 0
                    for q in range(4):
                        base = int(meta["selcol0"][w, q])
                        for (b2, t, col) in plan[w][q]:
                            if b2 != bb:
                                continue
                            nc.tensor.matmul(
                                out=acc[:],
                                lhsT=sels[q][:, (col - base) * P : (col - base + 1) * P],
                                rhs=msg[:, int(O_wq[w, q]) + t, :],
                                start=(ti == 0),
                                stop=(ti == nt - 1),
                            )
                            ti += 1
                    assert ti == nt
                    # epilogue: (acc + h_own) * dinv_dst + bias
                    nc.vector.tensor_tensor(
                        out=tmp_w[:, j, :],
                        in0=acc[:],
                        in1=gl[:, j, :],
                        op=mybir.AluOpType.add,
                    )
                    nc.scalar.activation(
                        out=osb_w[:, j, :],
                        in_=tmp_w[:, j, :],
                        func=mybir.ActivationFunctionType.Copy,
                        scale=dinv_sb[:, bb : bb + 1],
                    )
                    nc.vector.tensor_tensor(
                        out=osb_w[:, j, :],
                        in0=osb_w[:, j, :],
                        in1=bias_sb[:],
                        op=mybir.AluOpType.add,
                    )
                nc.sync.dma_start(
                    out=out_ext[w * WBLK * P : (w + 1) * WBLK * P, :].rearrange(
                        "(j p) f -> p j f", p=P
                    ),
                    in_=osb_w[:],
                )

    nc.finalize()
    return nc


def _run(inputs, trace=False, trace_cores=None):
    from concourse.bass_utils import run_bass_kernel_spmd

    meta, in_maps = _pack(**inputs)
    nc = _build_program(meta)
    res = run_bass_kernel_spmd(
        nc,
        in_maps,
        list(range(NCORES)),
        trace=trace,
        trace_cores=trace_cores,
    )
    out = np.empty((N_NODES, P), np.float32)
    for k in range(NCORES):
        out[k * NB : (k + 1) * NB] = np.asarray(res.results[k]["out"])[:NB]
    return out, res


def kernel(x, edge_index, weight, b):
    out, _ = _run(dict(x=x, edge_index=edge_index, weight=weight, b=b))
    return out


if __name__ == "__main__":
    rng = np.random.default_rng(0)
    n, e = 100000, 1600000
    x = rng.standard_normal((n, P), dtype=np.float32)
    ei = rng.integers(0, n, (2, e)).astype(np.int64)
    w = (rng.standard_normal((P, P)) / np.sqrt(P)).astype(np.float32)
    bb = (rng.standard_normal(P) * 0.02).astype(np.float32)
    out = kernel(x, ei, w, bb)
    print("out", out.shape, out.dtype)


# revision 42
# speedup vs baseline: 1.1037x; 1.1037x over previous
"""GCNConv (PyG-faithful, normalize=True, add_self_loops=True) on 8 Trainium2
NeuronCores via Bass/Tile.

Strategy (1D graph/data parallel, v2):
  - Nodes partitioned across 8 cores (12500 rows each, padded to 12544 = 98
    blocks of 128).
  - Phase A: each core computes h = (x @ W) * dinv_row in bf16 and writes it
    out in four "quarter" pieces (25/25/24/24 blocks). After each quarter an
    AllGather ships that quarter of every core into a per-quarter shared
    table chunk, so phase B can start gathering from chunk 0 while chunks
    1-3 are still in flight.
  - Phase B: each core owns 1/8 of the destinations. Edges (self-loops
    excluded - they are folded into the epilogue) are host-sorted by
    (dst-window, src-quarter, dst-block) and packed densely per
    (window, quarter) section; trailing pad indices are -1, which the SWDGE
    gather ucode trims, so pads cost no DMA packets. Four SWDGE queues (one
    per source quarter) gather 256B bf16 rows; host-precomputed one-hot fp8
    sel tiles feed TensorE matmuls that segment-sum each destination
    block's messages in PSUM. Epilogue per block: add the self-loop term
    (own h row), scale by dinv_dst, add bias.
"""

import sys

if "/opt/trn_rl_repo" not in sys.path:
    sys.path.insert(0, "/opt/trn_rl_repo")

import numpy as np

P = 128
NCORES = 8
WBLK = 7          # dst blocks per window
NW = 14           # windows (98 blocks / 7)
# Source rows are exchanged with TWO AllGathers over block-halves of each
# shard; gather "chunks" are (half x core-group) quadrants of the gathered
# tables, so chunks 0,1 are ready after the first AllGather and 2,3 after
# the second (earlier than a 4-way AllGather split, which serializes on the
# CC stream with per-op overhead).
HALF_BLKS = [50, 48]               # blocks per half
HALF_ROWS = [50 * P, 48 * P]       # 6400, 6144
HALF_OFF = [0, 50 * P]
N_NODES = 100000
NB = N_NODES // NCORES             # 12500
BLOCKS = 98
NBP = BLOCKS * P                   # 12544


def _pack(x, edge_index, weight, b):
    import ml_dtypes

    x = np.asarray(x, dtype=np.float32)
    ei = np.asarray(edge_index)
    weight = np.asarray(weight, dtype=np.float32)
    bias = np.asarray(b, dtype=np.float32).reshape(-1)

    n, nin = x.shape
    assert n == N_NODES and nin == P and weight.shape == (P, P)
    # chunk q = (half h, core-group cg): rows of cores cg*4..cg*4+3, half h
    chunk_rows = [4 * HALF_ROWS[0], 4 * HALF_ROWS[0], 4 * HALF_ROWS[1], 4 * HALF_ROWS[1]]

    src = ei[0].astype(np.int64)
    dst = ei[1].astype(np.int64)
    m = src.shape[0]

    deg = np.bincount(dst, minlength=n).astype(np.float32) + 1.0
    dinv = 1.0 / np.sqrt(deg)

    # --- per-edge coordinates (no self-loops; folded into epilogue)
    core = dst // NB
    dlc = dst - core * NB
    blk = dlc >> 7                   # dst block within core [0,98)
    dl = (dlc & 127).astype(np.int64)
    w = blk // WBLK                  # window [0,14)

    ks = src // NB
    ls = src - ks * NB
    hh = (ls >= HALF_OFF[1]).astype(np.int64)    # block-half of the source
    cg = (ks >= 4).astype(np.int64)              # core group
    q = 2 * hh + cg                              # chunk 0..3
    rel = (ks % 4) * np.take(HALF_ROWS, hh) + (ls - np.take(HALF_OFF, hh))
    assert rel.max() < 32768

    # order edges per core by (w, q, blk)
    sect = (w * 4 + q)               # (w,q) section id [0,56)
    key = (core * 56 + sect) * BLOCKS + blk
    order = np.argsort(key, kind="stable")
    karr = core[order]
    relarr = rel[order].astype(np.int16)
    dlarr = dl[order]
    sectarr = sect[order]
    blkarr = blk[order]

    # counts
    cnt_kwqb = np.bincount(
        (core * 56 + sect) * BLOCKS + blk, minlength=NCORES * 56 * BLOCKS
    ).reshape(NCORES, NW, 4, BLOCKS)
    cnt_kwq = cnt_kwqb.sum(axis=3)                       # [8, 14, 4]
    S_wq = -(-cnt_kwq.max(axis=0) // P)                  # [14, 4] tiles
    # per-core block start slot within its (w,q) section
    bstart = np.zeros((NCORES, NW, 4, BLOCKS), np.int64)
    for wi in range(NW):
        bl = slice(wi * WBLK, (wi + 1) * WBLK)
        c = cnt_kwqb[:, wi, :, bl]
        bstart[:, wi, :, bl] = np.cumsum(c, axis=2) - c

    # global tile layout + matmul plan
    sec0 = np.zeros((NW, 4), np.int64)      # global tile offset per section
    O_wq = np.zeros((NW, 4), np.int64)      # msg slot-tile offset in window
    t_total = 0
    for wi in range(NW):
        off = 0
        for qi in range(4):
            sec0[wi, qi] = t_total
            O_wq[wi, qi] = off
            t_total += int(S_wq[wi, qi])
            off += int(S_wq[wi, qi])
    jmax = int(S_wq.sum(axis=1).max())

    # matmul plan: per (w,q) ordered list of (b, t); col index global
    mm_col_of = np.full((NW, 4, BLOCKS, int(S_wq.max()) + 1), -1, np.int64)
    plan = [[[] for _ in range(4)] for _ in range(NW)]   # [(b, t, col)]
    selcol0 = np.zeros((NW, 4), np.int64)
    mm_total = 0
    ntiles_b = np.zeros(BLOCKS, np.int64)
    for wi in range(NW):
        for qi in range(4):
            selcol0[wi, qi] = mm_total
            for bb in range(wi * WBLK, (wi + 1) * WBLK):
                c = cnt_kwqb[:, wi, qi, bb]
                if c.max() == 0:
                    continue
                s = bstart[:, wi, qi, bb]
                e = s + c
                T0 = int((s // P).min())
                T1 = int((-(-e // P)).max())
                for t in range(T0, T1):
                    mm_col_of[wi, qi, bb, t] = mm_total
                    plan[wi][qi].append((bb, t, mm_total))
                    ntiles_b[bb] += 1
                    mm_total += 1
    assert (ntiles_b > 0).all()
    selmax = int(
        max(len(plan[wi][qi]) for wi in range(NW) for qi in range(4))
    )

    # per-core packed arrays
    gs = np.zeros(NCORES * NW * 4 * BLOCKS, np.int64)
    cr = cnt_kwqb.reshape(-1)
    gs[1:] = np.cumsum(cr)[:-1]
    kk = (karr * 56 + sectarr) * BLOCKS + blkarr
    rank_in_b = np.arange(m, dtype=np.int64) - gs[kk]
    wa = sectarr // 4
    qa = sectarr % 4
    slot = (
        bstart[karr, wa, qa, blkarr] + rank_in_b
    )                                                  # slot within section
    tile_in_sec = slot >> 7
    part = slot & 127
    gslot = sec0[wa, qa] * P + slot                    # global slot

    # pad slots gather row 0 of their chunk (sel column is zero, so the
    # value is discarded). Gathering pads costs ~4% extra packets but keeps
    # the per-core descriptor count equal to the static num_idxs, which the
    # SWDGE ring accounting requires (num_idxs_reg must match the actual
    # count, and per-core runtime registers proved unschedulable safely).
    idx_lin = np.zeros((NCORES, t_total * P), np.int16)
    idx_lin[karr, gslot] = relarr

    mmi = mm_col_of[wa, qa, blkarr, tile_in_sec]
    assert (mmi >= 0).all()
    sel_u8 = np.zeros((NCORES, P, mm_total * P), np.uint8)
    sel_u8[karr, part, mmi * P + dlarr] = 0x38         # fp8e4m3 1.0
    sel_pack = sel_u8.view(ml_dtypes.float8_e4m3)

    # wrap-16 + replicate to 128 partitions
    l16 = t_total * P // 16
    idx_w = idx_lin.reshape(NCORES, l16, 16).transpose(0, 2, 1)
    idx_pack = np.ascontiguousarray(np.tile(idx_w, (1, NCORES, 1)))

    # per-core xt (bf16, rows pre-scaled by dinv so phase A is matmul+copy)
    xt = np.zeros((NCORES, P, NBP), ml_dtypes.bfloat16)
    dinv_t = np.zeros((NCORES, P, BLOCKS), np.float32)
    for k in range(NCORES):
        xs = x[k * NB : (k + 1) * NB] * dinv[k * NB : (k + 1) * NB, None]
        xt[k, :, :NB] = xs.T.astype(ml_dtypes.bfloat16)
        dv = np.zeros(NBP, np.float32)
        dv[:NB] = dinv[k * NB : (k + 1) * NB]
        dinv_t[k] = dv.reshape(BLOCKS, P).T
    w_bf = np.ascontiguousarray(weight.astype(ml_dtypes.bfloat16))
    bias_rep = np.ascontiguousarray(np.tile(bias[None, :], (P, 1)))
    ident_u8 = np.zeros((P, P), np.uint8)
    ident_u8[np.arange(P), np.arange(P)] = 0x38       # fp8e4m3 identity
    ident = ident_u8.view(ml_dtypes.float8_e4m3)

    meta = dict(
        chunk_rows=chunk_rows, S_wq=S_wq, sec0=sec0, O_wq=O_wq,
        jmax=jmax, t_total=t_total, l16=l16, plan=plan, selcol0=selcol0,
        mm_total=mm_total, selmax=selmax, ntiles_b=ntiles_b,
    )
    in_maps = [
        {
            "xt": xt[k],
            "w_in": w_bf,
            "bias": bias_rep,
            "dinv": dinv_t[k],
            "idxp": idx_pack[k],
            "selp": sel_pack[k],
            "ident": ident,
        }
        for k in range(NCORES)
    ]
    return meta, in_maps


def _build_program(meta):
    from concourse import bass, bacc, mybir
    import concourse.tile as tile

    chunk_rows = meta["chunk_rows"]
    S_wq = meta["S_wq"]
    sec0 = meta["sec0"]
    O_wq = meta["O_wq"]
    jmax = meta["jmax"]
    l16 = meta["l16"]
    plan = meta["plan"]
    mm_total = meta["mm_total"]
    selmax = meta["selmax"]
    ntiles_b = meta["ntiles_b"]

    f32 = mybir.dt.float32
    bf16 = mybir.dt.bfloat16
    fp8 = mybir.dt.float8e4

    nc = bacc.Bacc(num_swdge_queues=4)
    xt_in = nc.declare_dram_parameter("xt", [P, NBP], bf16, isOutput=False)
    w_in = nc.declare_dram_parameter("w_in", [P, P], bf16, isOutput=False)
    bias_in = nc.declare_dram_parameter("bias", [P, P], f32, isOutput=False)
    dinv_in = nc.declare_dram_parameter("dinv", [P, BLOCKS], f32, isOutput=False)
    idx_in = nc.declare_dram_parameter("idxp", [P, l16], mybir.dt.int16, isOutput=False)
    sel_in = nc.declare_dram_parameter("selp", [P, mm_total * P], fp8, isOutput=False)
    ident_in = nc.declare_dram_parameter("ident", [P, P], fp8, isOutput=False)
    out_ext = nc.declare_dram_parameter("out", [NBP, P], f32, isOutput=True)

    hh = [nc.dram_tensor(f"h_h{i}", [HALF_ROWS[i], P], bf16) for i in range(2)]
    gh = [
        nc.dram_tensor(
            f"g_h{i}", [NCORES * HALF_ROWS[i], P], bf16, addr_space="Shared"
        )
        for i in range(2)
    ]

    with tile.TileContext(nc) as tc:
        with (
            tc.tile_pool(name="const", bufs=1) as cpool,
            tc.tile_pool(name="work", bufs=4) as wpool,
            tc.tile_pool(name="msgp", bufs=3) as mpool,
            tc.tile_pool(name="selp", bufs=8) as spool,
            tc.tile_pool(name="glp", bufs=2) as gpool,
            tc.tile_pool(name="outp", bufs=2) as opool,
            tc.tile_pool(name="psA", bufs=2, space="PSUM") as psA,
            tc.tile_pool(name="psB", bufs=4, space="PSUM") as psB,
        ):
            # constants / metadata
            w_sb = cpool.tile([P, P], bf16, tag="w")
            nc.sync.dma_start(out=w_sb[:], in_=w_in[:])
            bias_sb = cpool.tile([P, P], f32, tag="bias")
            nc.sync.dma_start(out=bias_sb[:], in_=bias_in[:])
            dinv_sb = cpool.tile([P, BLOCKS], f32, tag="dinv")
            nc.sync.dma_start(out=dinv_sb[:], in_=dinv_in[:])
            ident_sb = cpool.tile([P, P], fp8, tag="ident")
            nc.sync.dma_start(out=ident_sb[:], in_=ident_in[:])
            idx_sb = cpool.tile([P, l16], mybir.dt.int16, tag="idx")
            for i in range(4):
                s = l16 // 4
                e = l16 if i == 3 else (i + 1) * s
                nc.sync.dma_start(out=idx_sb[:, i * s : e], in_=idx_in[:, i * s : e])


            # ---- phase A: h = (dinv*x) @ W (dinv pre-folded into xt),
            # written per block-half, then AllGather each half.
            for i in range(2):
                nblk = HALF_BLKS[i]
                npc = -(-nblk // 6)
                lo = nblk // npc
                pieces = [lo + 1] * (nblk - lo * npc) + [lo] * (npc * (lo + 1) - nblk)
                assert sum(pieces) == nblk and max(pieces) <= 6
                loff = 0
                for pb in pieces:
                    xt_t = wpool.tile([P, 6 * P], bf16, tag="xt")
                    nc.sync.dma_start(
                        out=xt_t[:, : pb * P],
                        in_=xt_in[:, HALF_OFF[i] + loff : HALF_OFF[i] + loff + pb * P],
                    )
                    hbig = wpool.tile([P, 6, P], bf16, tag="hbig")
                    for j in range(pb):
                        ph = psA.tile([P, P], f32, tag="ph")
                        nc.tensor.matmul(
                            out=ph[:],
                            lhsT=xt_t[:, j * P : (j + 1) * P],
                            rhs=w_sb[:],
                            start=True,
                            stop=True,
                        )
                        nc.vector.tensor_copy(out=hbig[:, j, :], in_=ph[:])
                    nc.sync.dma_start(
                        out=hh[i][loff : loff + pb * P, :].rearrange(
                            "(j p) f -> p j f", p=P
                        ),
                        in_=hbig[:, :pb, :],
                    )
                    loff += pb * P
                nc.gpsimd.collective_compute(
                    "AllGather",
                    mybir.AluOpType.bypass,
                    replica_groups=[list(range(NCORES))],
                    ins=[hh[i][:]],
                    outs=[gh[i][:]],
                )

            # ---- phase B
            for w in range(NW):
                msg = mpool.tile([P, jmax, P], bf16, tag="msg")
                sels = [None] * 4
                for q in range(4):
                    swq = int(S_wq[w, q])
                    if swq == 0:
                        continue
                    s0 = int(sec0[w, q])
                    hi, cgi = q // 2, q % 2
                    gsrc = gh[hi][
                        cgi * 4 * HALF_ROWS[hi] : (cgi + 1) * 4 * HALF_ROWS[hi], :
                    ]
                    nc.gpsimd.dma_gather(
                        out_ap=msg[:, int(O_wq[w, q]) : int(O_wq[w, q]) + swq, :],
                        in_ap=gsrc,
                        idxs_ap=idx_sb[:, s0 * 8 : (s0 + swq) * 8],
                        num_idxs=swq * P,
                        num_idxs_reg=swq * P,
                        elem_size=P,
                        single_packet=False,
                        queue_num=q,
                    )
                    nmm = len(plan[w][q])
                    if nmm:
                        selw = spool.tile([P, selmax * P], fp8, tag="selw")
                        c0 = int(meta["selcol0"][w, q])
                        nc.scalar.dma_start(
                            out=selw[:, : nmm * P],
                            in_=sel_in[:, c0 * P : (c0 + nmm) * P],
                        )
                        sels[q] = selw

                # self-loop rows for this window (may straddle the halves)
                gl = gpool.tile([P, WBLK, P], bf16, tag="gl")
                r0, r1 = w * WBLK * P, (w + 1) * WBLK * P
                for i in range(2):
                    a = max(r0, HALF_OFF[i])
                    bq = min(r1, HALF_OFF[i] + HALF_ROWS[i])
                    if a >= bq:
                        continue
                    j0 = (a - r0) // P
                    j1 = (bq - r0) // P
                    nc.sync.dma_start(
                        out=gl[:, j0:j1, :],
                        in_=hh[i][a - HALF_OFF[i] : bq - HALF_OFF[i], :].rearrange(
                            "(j p) f -> p j f", p=P
                        ),
                    )

                tmp_w = opool.tile([P, WBLK, P], f32, tag="tmpw")
                osb_w = opool.tile([P, WBLK, P], f32, tag="osbw")
                for j, bb in enumerate(range(w * WBLK, (w + 1) * WBLK)):
                    nt = int(ntiles_b[bb])
                    acc = psB.tile([P, P], f32, tag="acc")
                    ti = 0
                    for q in range(4):
                        base = int(meta["selcol0"][w, q])
                        for (b2, t, col) in plan[w][q]:
                            if b2 != bb:
                                continue
                            nc.tensor.matmul(
                                out=acc[:],
                                lhsT=sels[q][:, (col - base) * P : (col - base + 1) * P],
                                rhs=msg[:, int(O_wq[w, q]) + t, :],
                                start=(ti == 0),
                                stop=(ti == nt - 1),
                            )
                            ti += 1
                    assert ti == nt
                    # epilogue: (acc + h_own) * dinv_dst + bias
                    nc.vector.tensor_tensor(
                        out=tmp_w[:, j, :],
                        in0=acc[:],
                        in1=gl[:, j, :],
                        op=mybir.AluOpType.add,
                    )
                    nc.scalar.activation(
                        out=osb_w[:, j, :],
                        in_=tmp_w[:, j, :],
                        func=mybir.ActivationFunctionType.Copy,
                        scale=dinv_sb[:, bb : bb + 1],
                    )
                    nc.vector.tensor_tensor(
                        out=osb_w[:, j, :],
                        in0=osb_w[:, j, :],
                        in1=bias_sb[:],
                        op=mybir.AluOpType.add,
                    )
                nc.sync.dma_start(
                    out=out_ext[w * WBLK * P : (w + 1) * WBLK * P, :].rearrange(
                        "(j p) f -> p j f", p=P
                    ),
                    in_=osb_w[:],
                )

    nc.finalize()
    return nc


def _run(inputs, trace=False, trace_cores=None):
    from concourse.bass_utils import run_bass_kernel_spmd

    meta, in_maps = _pack(**inputs)
    nc = _build_program(meta)
    res = run_bass_kernel_spmd(
        nc,
        in_maps,
        list(range(NCORES)),
        trace=trace,
        trace_cores=trace_cores,
    )
    out = np.empty((N_NODES, P), np.float32)
    for k in range(NCORES):
        out[k * NB : (k + 1) * NB] = np.asarray(res.results[k]["out"])[:NB]
    return out, res


def kernel(x, edge_index, weight, b):
    out, _ = _run(dict(x=x, edge_index=edge_index, weight=weight, b=b))
    return out


if __name__ == "__main__":
    rng = np.random.default_rng(0)
    n, e = 100000, 1600000
    x = rng.standard_normal((n, P), dtype=np.float32)
    ei = rng.integers(0, n, (2, e)).astype(np.int64)
    w = (rng.standard_normal((P, P)) / np.sqrt(P)).astype(np.float32)
    bb = (rng.standard_normal(P) * 0.02).astype(np.float32)
    out = kernel(x, ei, w, bb)
    print("out", out.shape, out.dtype)


# revision 45
# speedup vs baseline: 1.1438x; 1.0363x over previous
"""GCNConv (PyG-faithful, normalize=True, add_self_loops=True) on 8 Trainium2
NeuronCores via Bass/Tile.

Strategy (1D graph/data parallel, v2):
  - Nodes partitioned across 8 cores (12500 rows each, padded to 12544 = 98
    blocks of 128).
  - Phase A: each core computes h = (x @ W) * dinv_row in bf16 and writes it
    out in four "quarter" pieces (25/25/24/24 blocks). After each quarter an
    AllGather ships that quarter of every core into a per-quarter shared
    table chunk, so phase B can start gathering from chunk 0 while chunks
    1-3 are still in flight.
  - Phase B: each core owns 1/8 of the destinations. Edges (self-loops
    excluded - they are folded into the epilogue) are host-sorted by
    (dst-window, src-quarter, dst-block) and packed densely per
    (window, quarter) section; trailing pad indices are -1, which the SWDGE
    gather ucode trims, so pads cost no DMA packets. Four SWDGE queues (one
    per source quarter) gather 256B bf16 rows; host-precomputed one-hot fp8
    sel tiles feed TensorE matmuls that segment-sum each destination
    block's messages in PSUM. Epilogue per block: add the self-loop term
    (own h row), scale by dinv_dst, add bias.
"""

import sys

if "/opt/trn_rl_repo" not in sys.path:
    sys.path.insert(0, "/opt/trn_rl_repo")

import numpy as np

P = 128
NCORES = 8
WBLK = 7          # dst blocks per window
NW = 14           # windows (98 blocks / 7)
# Source rows are exchanged with TWO AllGathers over block-halves of each
# shard; gather "chunks" are (half x core-group) quadrants of the gathered
# tables, so chunks 0,1 are ready after the first AllGather and 2,3 after
# the second (earlier than a 4-way AllGather split, which serializes on the
# CC stream with per-op overhead).
HALF_BLKS = [50, 48]               # blocks per half
HALF_ROWS = [50 * P, 48 * P]       # 6400, 6144
HALF_OFF = [0, 50 * P]
N_NODES = 100000
NB = N_NODES // NCORES             # 12500
BLOCKS = 98
NBP = BLOCKS * P                   # 12544


def _pack(x, edge_index, weight, b):
    import ml_dtypes

    x = np.asarray(x, dtype=np.float32)
    ei = np.asarray(edge_index)
    weight = np.asarray(weight, dtype=np.float32)
    bias = np.asarray(b, dtype=np.float32).reshape(-1)

    n, nin = x.shape
    assert n == N_NODES and nin == P and weight.shape == (P, P)
    # chunk q = (half h, core-group cg): rows of cores cg*4..cg*4+3, half h
    chunk_rows = [4 * HALF_ROWS[0], 4 * HALF_ROWS[0], 4 * HALF_ROWS[1], 4 * HALF_ROWS[1]]

    src = ei[0].astype(np.int64)
    dst = ei[1].astype(np.int64)
    m = src.shape[0]

    deg = np.bincount(dst, minlength=n).astype(np.float32) + 1.0
    dinv = 1.0 / np.sqrt(deg)

    # --- per-edge coordinates (no self-loops; folded into epilogue)
    core = dst // NB
    dlc = dst - core * NB
    blk = dlc >> 7                   # dst block within core [0,98)
    dl = (dlc & 127).astype(np.int64)
    w = blk // WBLK                  # window [0,14)

    ks = src // NB
    ls = src - ks * NB
    hh = (ls >= HALF_OFF[1]).astype(np.int64)    # block-half of the source
    cg = (ks >= 4).astype(np.int64)              # core group
    q = 2 * hh + cg                              # chunk 0..3
    rel = (ks % 4) * np.take(HALF_ROWS, hh) + (ls - np.take(HALF_OFF, hh))
    assert rel.max() < 32768

    # order edges per core by (w, q, blk)
    sect = (w * 4 + q)               # (w,q) section id [0,56)
    key = (core * 56 + sect) * BLOCKS + blk
    order = np.argsort(key, kind="stable")
    karr = core[order]
    relarr = rel[order].astype(np.int16)
    dlarr = dl[order]
    sectarr = sect[order]
    blkarr = blk[order]

    # counts
    cnt_kwqb = np.bincount(
        (core * 56 + sect) * BLOCKS + blk, minlength=NCORES * 56 * BLOCKS
    ).reshape(NCORES, NW, 4, BLOCKS)
    cnt_kwq = cnt_kwqb.sum(axis=3)                       # [8, 14, 4]
    S_wq = -(-cnt_kwq.max(axis=0) // P)                  # [14, 4] tiles
    # per-core block start slot within its (w,q) section
    bstart = np.zeros((NCORES, NW, 4, BLOCKS), np.int64)
    for wi in range(NW):
        bl = slice(wi * WBLK, (wi + 1) * WBLK)
        c = cnt_kwqb[:, wi, :, bl]
        bstart[:, wi, :, bl] = np.cumsum(c, axis=2) - c

    # global tile layout + matmul plan
    sec0 = np.zeros((NW, 4), np.int64)      # global tile offset per section
    O_wq = np.zeros((NW, 4), np.int64)      # msg slot-tile offset in window
    t_total = 0
    for wi in range(NW):
        off = 0
        for qi in range(4):
            sec0[wi, qi] = t_total
            O_wq[wi, qi] = off
            t_total += int(S_wq[wi, qi])
            off += int(S_wq[wi, qi])
    jmax = int(S_wq.sum(axis=1).max())

    # matmul plan: per (w,q) ordered list of (b, t); col index global
    mm_col_of = np.full((NW, 4, BLOCKS, int(S_wq.max()) + 1), -1, np.int64)
    plan = [[[] for _ in range(4)] for _ in range(NW)]   # [(b, t, col)]
    selcol0 = np.zeros((NW, 4), np.int64)
    mm_total = 0
    ntiles_b = np.zeros(BLOCKS, np.int64)
    for wi in range(NW):
        for qi in range(4):
            selcol0[wi, qi] = mm_total
            for bb in range(wi * WBLK, (wi + 1) * WBLK):
                c = cnt_kwqb[:, wi, qi, bb]
                if c.max() == 0:
                    continue
                s = bstart[:, wi, qi, bb]
                e = s + c
                T0 = int((s // P).min())
                T1 = int((-(-e // P)).max())
                for t in range(T0, T1):
                    mm_col_of[wi, qi, bb, t] = mm_total
                    plan[wi][qi].append((bb, t, mm_total))
                    ntiles_b[bb] += 1
                    mm_total += 1
    assert (ntiles_b > 0).all()
    selmax = int(
        max(len(plan[wi][qi]) for wi in range(NW) for qi in range(4))
    )

    # per-core packed arrays
    gs = np.zeros(NCORES * NW * 4 * BLOCKS, np.int64)
    cr = cnt_kwqb.reshape(-1)
    gs[1:] = np.cumsum(cr)[:-1]
    kk = (karr * 56 + sectarr) * BLOCKS + blkarr
    rank_in_b = np.arange(m, dtype=np.int64) - gs[kk]
    wa = sectarr // 4
    qa = sectarr % 4
    slot = (
        bstart[karr, wa, qa, blkarr] + rank_in_b
    )                                                  # slot within section
    tile_in_sec = slot >> 7
    part = slot & 127
    gslot = sec0[wa, qa] * P + slot                    # global slot

    # pad slots gather row 0 of their chunk (sel column is zero, so the
    # value is discarded). Gathering pads costs ~4% extra packets but keeps
    # the per-core descriptor count equal to the static num_idxs, which the
    # SWDGE ring accounting requires (num_idxs_reg must match the actual
    # count, and per-core runtime registers proved unschedulable safely).
    idx_lin = np.zeros((NCORES, t_total * P), np.int16)
    idx_lin[karr, gslot] = relarr

    mmi = mm_col_of[wa, qa, blkarr, tile_in_sec]
    assert (mmi >= 0).all()
    sel_u8 = np.zeros((NCORES, P, mm_total * P), np.uint8)
    sel_u8[karr, part, mmi * P + dlarr] = 0x38         # fp8e4m3 1.0
    sel_pack = sel_u8.view(ml_dtypes.float8_e4m3)

    # wrap-16 + replicate to 128 partitions
    l16 = t_total * P // 16
    idx_w = idx_lin.reshape(NCORES, l16, 16).transpose(0, 2, 1)
    idx_pack = np.ascontiguousarray(np.tile(idx_w, (1, NCORES, 1)))

    # per-core xt (bf16, rows pre-scaled by dinv so phase A is matmul+copy)
    xt = np.zeros((NCORES, P, NBP), ml_dtypes.bfloat16)
    dinv_t = np.zeros((NCORES, P, BLOCKS), np.float32)
    for k in range(NCORES):
        xs = x[k * NB : (k + 1) * NB] * dinv[k * NB : (k + 1) * NB, None]
        xt[k, :, :NB] = xs.T.astype(ml_dtypes.bfloat16)
        dv = np.zeros(NBP, np.float32)
        dv[:NB] = dinv[k * NB : (k + 1) * NB]
        dinv_t[k] = dv.reshape(BLOCKS, P).T
    w_bf = np.ascontiguousarray(weight.astype(ml_dtypes.bfloat16))
    bias_rep = np.ascontiguousarray(np.tile(bias[None, :], (P, 1)))
    ident_u8 = np.zeros((P, P), np.uint8)
    ident_u8[np.arange(P), np.arange(P)] = 0x38       # fp8e4m3 identity
    ident = ident_u8.view(ml_dtypes.float8_e4m3)

    meta = dict(
        chunk_rows=chunk_rows, S_wq=S_wq, sec0=sec0, O_wq=O_wq,
        jmax=jmax, t_total=t_total, l16=l16, plan=plan, selcol0=selcol0,
        mm_total=mm_total, selmax=selmax, ntiles_b=ntiles_b,
    )
    in_maps = [
        {
            "xt": xt[k],
            "w_in": w_bf,
            "bias": bias_rep,
            "dinv": dinv_t[k],
            "idxp": idx_pack[k],
            "selp": sel_pack[k],
            "ident": ident,
        }
        for k in range(NCORES)
    ]
    return meta, in_maps


def _build_program(meta):
    from concourse import bass, bacc, mybir
    import concourse.tile as tile

    chunk_rows = meta["chunk_rows"]
    S_wq = meta["S_wq"]
    sec0 = meta["sec0"]
    O_wq = meta["O_wq"]
    jmax = meta["jmax"]
    l16 = meta["l16"]
    plan = meta["plan"]
    mm_total = meta["mm_total"]
    selmax = meta["selmax"]
    ntiles_b = meta["ntiles_b"]

    f32 = mybir.dt.float32
    bf16 = mybir.dt.bfloat16
    fp8 = mybir.dt.float8e4

    nc = bacc.Bacc(num_swdge_queues=4)
    xt_in = nc.declare_dram_parameter("xt", [P, NBP], bf16, isOutput=False)
    w_in = nc.declare_dram_parameter("w_in", [P, P], bf16, isOutput=False)
    bias_in = nc.declare_dram_parameter("bias", [P, P], f32, isOutput=False)
    dinv_in = nc.declare_dram_parameter("dinv", [P, BLOCKS], f32, isOutput=False)
    idx_in = nc.declare_dram_parameter("idxp", [P, l16], mybir.dt.int16, isOutput=False)
    sel_in = nc.declare_dram_parameter("selp", [P, mm_total * P], fp8, isOutput=False)
    ident_in = nc.declare_dram_parameter("ident", [P, P], fp8, isOutput=False)
    out_ext = nc.declare_dram_parameter("out", [NBP, P], f32, isOutput=True)

    hh = [nc.dram_tensor(f"h_h{i}", [HALF_ROWS[i], P], bf16) for i in range(2)]
    gh = [
        nc.dram_tensor(
            f"g_h{i}", [NCORES * HALF_ROWS[i], P], bf16, addr_space="Shared"
        )
        for i in range(2)
    ]

    with tile.TileContext(nc) as tc:
        with (
            tc.tile_pool(name="const", bufs=1) as cpool,
            tc.tile_pool(name="work", bufs=4) as wpool,
            tc.tile_pool(name="msgp", bufs=3) as mpool,
            tc.tile_pool(name="selp", bufs=8) as spool,
            tc.tile_pool(name="glp", bufs=2) as gpool,
            tc.tile_pool(name="outp", bufs=2) as opool,
            tc.tile_pool(name="psA", bufs=2, space="PSUM") as psA,
            tc.tile_pool(name="psB", bufs=4, space="PSUM") as psB,
        ):
            # constants / metadata
            w_sb = cpool.tile([P, P], bf16, tag="w")
            nc.sync.dma_start(out=w_sb[:], in_=w_in[:])
            bias_sb = cpool.tile([P, P], f32, tag="bias")
            nc.sync.dma_start(out=bias_sb[:], in_=bias_in[:])
            dinv_sb = cpool.tile([P, BLOCKS], f32, tag="dinv")
            nc.sync.dma_start(out=dinv_sb[:], in_=dinv_in[:])
            ident_sb = cpool.tile([P, P], fp8, tag="ident")
            nc.sync.dma_start(out=ident_sb[:], in_=ident_in[:])
            idx_sb = cpool.tile([P, l16], mybir.dt.int16, tag="idx")
            for i in range(4):
                s = l16 // 4
                e = l16 if i == 3 else (i + 1) * s
                nc.sync.dma_start(out=idx_sb[:, i * s : e], in_=idx_in[:, i * s : e])


            # ---- phase A: h = (dinv*x) @ W (dinv pre-folded into xt),
            # written per block-half, then AllGather each half.
            for i in range(2):
                nblk = HALF_BLKS[i]
                npc = -(-nblk // 6)
                lo = nblk // npc
                pieces = [lo + 1] * (nblk - lo * npc) + [lo] * (npc * (lo + 1) - nblk)
                assert sum(pieces) == nblk and max(pieces) <= 6
                loff = 0
                for pb in pieces:
                    xt_t = wpool.tile([P, 6 * P], bf16, tag="xt")
                    nc.sync.dma_start(
                        out=xt_t[:, : pb * P],
                        in_=xt_in[:, HALF_OFF[i] + loff : HALF_OFF[i] + loff + pb * P],
                    )
                    hbig = wpool.tile([P, 6, P], bf16, tag="hbig")
                    for j in range(pb):
                        ph = psA.tile([P, P], f32, tag="ph")
                        nc.tensor.matmul(
                            out=ph[:],
                            lhsT=xt_t[:, j * P : (j + 1) * P],
                            rhs=w_sb[:],
                            start=True,
                            stop=True,
                        )
                        nc.vector.tensor_copy(out=hbig[:, j, :], in_=ph[:])
                    # h writes go out on the scalar HWDGE queue (idle during
                    # phase A) so they don't serialize behind the next piece's
                    # xt load on the sync queue (phase A gates the AllGathers)
                    nc.scalar.dma_start(
                        out=hh[i][loff : loff + pb * P, :].rearrange(
                            "(j p) f -> p j f", p=P
                        ),
                        in_=hbig[:, :pb, :],
                    )
                    loff += pb * P
                nc.gpsimd.collective_compute(
                    "AllGather",
                    mybir.AluOpType.bypass,
                    replica_groups=[list(range(NCORES))],
                    ins=[hh[i][:]],
                    outs=[gh[i][:]],
                )

            # ---- phase B
            for w in range(NW):
                msg = mpool.tile([P, jmax, P], bf16, tag="msg")
                sels = [None] * 4
                for q in range(4):
                    swq = int(S_wq[w, q])
                    if swq == 0:
                        continue
                    s0 = int(sec0[w, q])
                    hi, cgi = q // 2, q % 2
                    gsrc = gh[hi][
                        cgi * 4 * HALF_ROWS[hi] : (cgi + 1) * 4 * HALF_ROWS[hi], :
                    ]
                    # split the first window's section in two gather calls:
                    # descriptor generation (~30us for a full section) gates
                    # the queue's first drain right after the AllGather lands
                    parts = [swq] if w > 0 else [swq // 2, swq - swq // 2]
                    p0 = 0
                    for sp in parts:
                        if sp == 0:
                            continue
                        nc.gpsimd.dma_gather(
                            out_ap=msg[
                                :,
                                int(O_wq[w, q]) + p0 : int(O_wq[w, q]) + p0 + sp,
                                :,
                            ],
                            in_ap=gsrc,
                            idxs_ap=idx_sb[:, (s0 + p0) * 8 : (s0 + p0 + sp) * 8],
                            num_idxs=sp * P,
                            num_idxs_reg=sp * P,
                            elem_size=P,
                            single_packet=False,
                            queue_num=q,
                        )
                        p0 += sp
                    nmm = len(plan[w][q])
                    if nmm:
                        selw = spool.tile([P, selmax * P], fp8, tag="selw")
                        c0 = int(meta["selcol0"][w, q])
                        nc.scalar.dma_start(
                            out=selw[:, : nmm * P],
                            in_=sel_in[:, c0 * P : (c0 + nmm) * P],
                        )
                        sels[q] = selw

                # self-loop rows for this window (may straddle the halves)
                gl = gpool.tile([P, WBLK, P], bf16, tag="gl")
                r0, r1 = w * WBLK * P, (w + 1) * WBLK * P
                for i in range(2):
                    a = max(r0, HALF_OFF[i])
                    bq = min(r1, HALF_OFF[i] + HALF_ROWS[i])
                    if a >= bq:
                        continue
                    j0 = (a - r0) // P
                    j1 = (bq - r0) // P
                    nc.sync.dma_start(
                        out=gl[:, j0:j1, :],
                        in_=hh[i][a - HALF_OFF[i] : bq - HALF_OFF[i], :].rearrange(
                            "(j p) f -> p j f", p=P
                        ),
                    )

                tmp_w = opool.tile([P, WBLK, P], f32, tag="tmpw")
                osb_w = opool.tile([P, WBLK, P], f32, tag="osbw")
                for j, bb in enumerate(range(w * WBLK, (w + 1) * WBLK)):
                    nt = int(ntiles_b[bb])
                    acc = psB.tile([P, P], f32, tag="acc")
                    ti = 0
                    for q in range(4):
                        base = int(meta["selcol0"][w, q])
                        for (b2, t, col) in plan[w][q]:
                            if b2 != bb:
                                continue
                            nc.tensor.matmul(
                                out=acc[:],
                                lhsT=sels[q][:, (col - base) * P : (col - base + 1) * P],
                                rhs=msg[:, int(O_wq[w, q]) + t, :],
                                start=(ti == 0),
                                stop=(ti == nt - 1),
                            )
                            ti += 1
                    assert ti == nt
                    # epilogue: (acc + h_own) * dinv_dst + bias
                    nc.vector.tensor_tensor(
                        out=tmp_w[:, j, :],
                        in0=acc[:],
                        in1=gl[:, j, :],
                        op=mybir.AluOpType.add,
                    )
                    nc.scalar.activation(
                        out=osb_w[:, j, :],
                        in_=tmp_w[:, j, :],
                        func=mybir.ActivationFunctionType.Copy,
                        scale=dinv_sb[:, bb : bb + 1],
                    )
                    nc.vector.tensor_tensor(
                        out=osb_w[:, j, :],
                        in0=osb_w[:, j, :],
                        in1=bias_sb[:],
                        op=mybir.AluOpType.add,
                    )
                nc.sync.dma_start(
                    out=out_ext[w * WBLK * P : (w + 1) * WBLK * P, :].rearrange(
                        "(j p) f -> p j f", p=P
                    ),
                    in_=osb_w[:],
                )

    nc.finalize()
    return nc


def _run(inputs, trace=False, trace_cores=None):
    from concourse.bass_utils import run_bass_kernel_spmd

    meta, in_maps = _pack(**inputs)
    nc = _build_program(meta)
    res = run_bass_kernel_spmd(
        nc,
        in_maps,
        list(range(NCORES)),
        trace=trace,
        trace_cores=trace_cores,
    )
    out = np.empty((N_NODES, P), np.float32)
    for k in range(NCORES):
        out[k * NB : (k + 1) * NB] = np.asarray(res.results[k]["out"])[:NB]
    return out, res


def kernel(x, edge_index, weight, b):
    out, _ = _run(dict(x=x, edge_index=edge_index, weight=weight, b=b))
    return out


if __name__ == "__main__":
    rng = np.random.default_rng(0)
    n, e = 100000, 1600000
    x = rng.standard_normal((n, P), dtype=np.float32)
    ei = rng.integers(0, n, (2, e)).astype(np.int64)
    w = (rng.standard_normal((P, P)) / np.sqrt(P)).astype(np.float32)
    bb = (rng.standard_normal(P) * 0.02).astype(np.float32)
    out = kernel(x, ei, w, bb)
    print("out", out.shape, out.dtype)
